# revision 1
# baseline (speedup 1.0000x reference)
"""Trainium2 Bass kernel for nn_DiTBlock_77979426226864.

Sharding: 8 cores = (batch b in 0..3) x (sequence half in 0..1). Each core
gets a zero-padded extended input x_ext [512, 64+2048+64] and computes its
2048-position output slice. The MinGRU scans use the 64-position halo in
place of a cross-core carry exchange (the per-step decay sigmoid(-g) makes
the truncation error far below fp32 noise; validated against the
reference). The depthwise-3 convs use a 1-column halo on the proj output
with per-core edge masking.

MinGRU runs as the linear recurrence H_t = c_t*H_{t-1} + b_t with
c = sigmoid(-g'), b = sigmoid(g')*gfunc(h'), gfunc(h) = max(h+0.5,
sigmoid(h)), on the DVE tensor_tensor_scan instruction. The backward
direction uses reversed-AP local scans per 512-chunk plus a carry-chain
fixup (blocked scan). Magnitude-preserving norms fold into per-partition
ACT scales; conditioning scale folds into lhsT columns; shifts fold into
per-partition ACT biases. All compute is on-device; the host only
pads/slices/reshapes for sharding.
"""
import os
import sys
import functools

for _p in ("/opt/trn_rl_repo", "/root/.axon_site"):
    if _p not in sys.path and os.path.isdir(_p):
        sys.path.insert(0, _p)

import numpy as np

import concourse.bass as bass  # noqa: E402
import concourse.bacc as bacc  # noqa: E402
import concourse.tile as tile  # noqa: E402
from concourse import mybir  # noqa: E402
from concourse.bass_utils import run_bass_kernel_spmd  # noqa: E402

F32 = mybir.dt.float32
AF = mybir.ActivationFunctionType
OP = mybir.AluOpType

B, D, L = 4, 512, 4096
C = 256
O = 512
OV = 64
LLOC = L // 2
LEXT = OV + LLOC + OV          # 2176
NG = D // 128                  # 4
CW = 512
HCOL0 = OV - 1                 # ext col of H/X2/Rchn col 0

A_CHUNKS = [(0, OV), (OV, OV + 512), (OV + 512, OV + 1024),
            (OV + 1024, OV + 1536), (OV + 1536, OV + 2048),
            (OV + 2048, LEXT)]
C1_CHUNKS = [(OV - 1, OV + 511), (OV + 511, OV + 1023),
             (OV + 1023, OV + 1535), (OV + 1535, OV + 2047),
             (OV + 2047, OV + 2049)]
C2B_CHUNKS = [(OV, OV + 512), (OV + 512, OV + 1024),
              (OV + 1024, OV + 1536), (OV + 1536, OV + 2048)]

MM_DT = None
F32R = mybir.dt.float32r   # rounded matmul: 1.5 cyc/row vs 2.0
# CoreSim lacks Silu; set env KERNEL_SIM_SAFE=1 to substitute Sigmoid (for
# simulator debugging only -- numerics checked against a matching model).
GATE_FN = (AF.Sigmoid if os.environ.get("KERNEL_SIM_SAFE") else AF.Silu)
# hardware bisection: PRE | A | C1 | FULL
STAGE = os.environ.get("KERNEL_STAGE", "FULL")

MAIN_WS = ["fore_W", "back_W", "seq_out_W", "proj_in_W", "pwh_W",
           "pwg_W", "chn_out_W"]
COND_WS = ["sm_scale_W", "sm_shift_W", "sm_alpha_W",
           "cm_scale_W", "cm_shift_W", "cm_alpha_W"]
GAIN_WS = ["sm_scale_g", "sm_shift_g", "sm_alpha_g",
           "cm_scale_g", "cm_shift_g", "cm_alpha_g"]


def _mm(ap):
    if MM_DT is None:
        return ap
    return ap.bitcast(MM_DT)


def build_program():
    nc = bacc.Bacc("TRN2", target_bir_lowering=False, debug=False,
                   num_devices=8)

    x_in = nc.dram_tensor("x_ext", [D, LEXT], F32, kind="ExternalInput")
    c_in = nc.dram_tensor("c_row", [1, C], F32, kind="ExternalInput")
    sel_in = nc.dram_tensor("sel", [128, 2], F32, kind="ExternalInput")
    w_shapes = {"fore_W": [2 * O, D], "back_W": [2 * O, D],
                "seq_out_W": [D, 2 * O], "proj_in_W": [D, D],
                "pwh_W": [2 * D, D], "pwg_W": [2 * D, D],
                "chn_out_W": [D, 2 * D]}
    for n in COND_WS:
        w_shapes[n] = [D, C]
    w_in = {n: nc.dram_tensor(n, s, F32, kind="ExternalInput")
            for n, s in w_shapes.items()}
    for n in GAIN_WS:
        w_in[n] = nc.dram_tensor(n, [1, 1], F32, kind="ExternalInput")
    w_in["dwh_W"] = nc.dram_tensor("dwh_W", [D, 3], F32,
                                   kind="ExternalInput")
    w_in["dwg_W"] = nc.dram_tensor("dwg_W", [D, 3], F32,
                                   kind="ExternalInput")
    out_d = nc.dram_tensor("out", [D, LLOC], F32, kind="ExternalOutput")

    ident_d = nc.inline_tensor(np.eye(128, dtype=np.float32), name="identm")
    onesc_d = nc.inline_tensor(np.ones((128, 1), np.float32),
                               name="onescol")
    onesr_d = nc.inline_tensor(np.ones((1, 128), np.float32),
                               name="onesrow")

    with tile.TileContext(nc) as tc:
        _emit(nc, tc, x_in, c_in, sel_in, w_in, out_d,
              ident_d, onesc_d, onesr_d)
    nc.compile()
    return nc


def _emit(nc, tc, x_in, c_in, sel_in, w_in, out_d,
          ident_d, onesc_d, onesr_d):

    def xdram(lo, hi):
        return x_in.ap()[:, lo:hi].rearrange("(g p) l -> p g l", p=128)

    # ---------------- pool stack (strict LIFO) ----------------
    pconst = tc.alloc_tile_pool(name="constp", bufs=1)
    pvec = tc.alloc_tile_pool(name="vecp", bufs=1)
    psum = tc.alloc_tile_pool(name="psump", bufs=1, space="PSUM")
    pdram = tc.alloc_tile_pool(name="dramp", bufs=1, space="DRAM")
    pbig = tc.alloc_tile_pool(name="bigp", bufs=1)
    pW4k = tc.alloc_tile_pool(name="w4kp", bufs=1)
    pW2k = tc.alloc_tile_pool(name="w2kp", bufs=1)
    prows = tc.alloc_tile_pool(name="rowsp", bufs=1)

    def T(pool, shape, tag, bufs=1, dt=F32):
        return pool.tile(shape, dt, tag=tag, bufs=bufs, name=tag)

    def PS(shape, small=False):
        return psum.tile(shape, F32, tag="psSMALL" if small else "psBIG",
                         bufs=2 if small else 6,
                         name="psS" if small else "psB")

    # DRAM scratch
    pspill_t = pdram.tile([D, 2048], F32, tag="pspill", name="pspill")
    x2spill = pdram.tile([D, 2050], F32, tag="x2spill", name="x2spill")

    # ---------------- constants ----------------
    ident = T(pconst, [128, 128], "ident")
    nc.sync.dma_start(ident[:], ident_d.ap())
    onesc = T(pconst, [128, 1], "onesc")
    nc.sync.dma_start(onesc[:], onesc_d.ap())
    onesr = T(pconst, [1, 128], "onesr")
    nc.sync.dma_start(onesr[:], onesr_d.ap())
    zeros = T(pconst, [128, CW], "zeros")
    nc.gpsimd.memset(zeros[:], 0.0)
    eps = T(pconst, [128, 1], "eps")
    nc.gpsimd.memset(eps[:], 1e-4)
    sel = T(pconst, [128, 2], "sel")
    nc.sync.dma_start(sel[:], sel_in.ap())
    selL, selR = sel[:, 0:1], sel[:, 1:2]
    crow = T(pconst, [1, C], "crow")
    nc.sync.dma_start(crow[:], c_in.ap())
    dwh = T(pconst, [128, NG, 3], "dwh")
    nc.sync.dma_start(dwh[:], w_in["dwh_W"].ap().rearrange(
        "(g p) k -> p g k", p=128))
    dwg = T(pconst, [128, NG, 3], "dwg")
    nc.sync.dma_start(dwg[:], w_in["dwg_W"].ap().rearrange(
        "(g p) k -> p g k", p=128))
    dwhn = T(pconst, [128, NG, 3], "dwhn")
    dwgn = T(pconst, [128, NG, 3], "dwgn")

    # big shared-slot tensors: Hf, Hb now; Rchn reuses a slot later
    Hf = T(pbig, [128, NG, 2050], "big", bufs=2, dt=F32R)
    Hb = T(pbig, [128, NG, 2050], "big", bufs=2, dt=F32R)

    # rows: one slot, sqrt+recip applied in place; rowB reuses after A1
    rowA = T(prows, [1, LEXT], "rows", bufs=1)
    rowAi = rowA

    # ---------------- pass-A transient pool ----------------
    pA = tc.alloc_tile_pool(name="pAp", bufs=1)
    pPre = tc.alloc_tile_pool(name="pPrep", bufs=1)

    def natload(name, mq, mspan):
        # load m-tile rows [mq, mq+mspan) of weight `name` natural layout
        cols = w_in[name].shape[1]
        t = T(pPre, [128, mspan, cols], "nat", bufs=2)
        nc.sync.dma_start(
            t[:], w_in[name].ap()[mq * 128:(mq + mspan) * 128, :].rearrange(
                "(m p) k -> p m k", p=128))
        return t

    def sq_accum(src, n2col):
        sq = T(pPre, [128, 1024], "sqscr", bufs=1)
        nc.scalar.activation(sq[:, 0:src.shape[-1]], src, AF.Square,
                             accum_out=n2col)
        return sq

    # ---------------- pass A0: pixel-norm row ----------------
    if STAGE == "PRE":
        junk = T(pA, [128, NG, CW], "xA", bufs=2)
        nc.gpsimd.memset(junk[:], 0.0)
        for j in range(4):
            nc.sync.dma_start(
                out_d.ap()[:, j * 512:(j + 1) * 512].rearrange(
                    "(g p) l -> p g l", p=128), junk[:])
    _stageA = STAGE in ("A", "C1", "FULL")
    for (lo, hi) in (A_CHUNKS if _stageA else []):
        cw = hi - lo
        xt = T(pA, [128, NG, CW], "xA", bufs=1, dt=F32R)
        nc.sync.dma_start(xt[:, :, 0:cw], xdram(lo, hi).bitcast(F32R))
        nc.scalar.activation(xt[:, :, 0:cw], xt[:, :, 0:cw], AF.Square)
        rps = PS([1, CW], small=True)
        for g in range(NG):
            nc.tensor.matmul(rps[:, 0:cw], onesc[:],
                             xt[:, g, 0:cw].bitcast(F32),
                             start=(g == 0), stop=(g == NG - 1))
        nc.scalar.copy(rowA[:, lo:hi], rps[:, 0:cw])
    nc.scalar.activation(rowAi[:], rowA[:], AF.Sqrt,
                         bias=eps[0:1, :], scale=1.0 / D)
    nc.vector.reciprocal(rowAi[:], rowAi[:])



    # ---------------- weight prep helper ----------------
    invn = {}
    conds = {}

    def prep_weight(name, pool, ltag, m_tiles, k_tiles):
        """Load name, compute invn, transpose into k_tiles lhsT tiles."""
        inv = T(pvec, [128, m_tiles], f"invn_{name}")
        n2 = T(pPre, [128, 8], "n2", bufs=2)
        std = T(pPre, [128, 8], "nstd", bufs=2)
        lhsT = [T(pool, [128, m_tiles * 128], ltag, bufs=pool._ltbufs,
                  dt=F32R)
                for _ in range(k_tiles)]
        step = 2 if w_in[name].shape[1] <= 512 else 1
        for mq in range(0, m_tiles, step):
            mspan = min(step, m_tiles - mq)
            natw = natload(name, mq, mspan)
            for j in range(mspan):
                sq_accum(natw[:, j, :], n2[:, mq + j:mq + j + 1])
                for k in range(k_tiles):
                    tp = PS([128, 128])
                    nc.tensor.transpose(
                        tp[:, 0:128],
                        natw[:, j, k * 128:(k + 1) * 128], ident[:])
                    nc.vector.tensor_copy(
                        lhsT[k][:, (mq + j) * 128:(mq + j + 1) * 128],
                        tp[:, 0:128])
        nc.scalar.activation(std[:, 0:m_tiles], n2[:, 0:m_tiles], AF.Sqrt)
        nc.vector.reciprocal(inv[:], std[:, 0:m_tiles])
        invn[name] = inv
        return lhsT

    pW4k._ltbufs = 8
    pW2k._ltbufs = 12
    lt_fore = prep_weight("fore_W", pW4k, "lt4k", 8, 4)
    lt_back = prep_weight("back_W", pW4k, "lt4k", 8, 4)
    lt_seq = prep_weight("seq_out_W", pW2k, "lt2k", 4, 8)

    # ---------------- conditioning ----------------
    cbc_ps = PS([128, C], small=True)
    nc.tensor.matmul(cbc_ps[:], _mm(onesr[:]), _mm(crow[:]),
                     start=True, stop=True)
    cbc = T(pvec, [128, C], "cbc")
    nc.scalar.copy(cbc[:], cbc_ps[:])
    gbc = {}
    for gname in GAIN_WS:
        grow = T(pconst, [1, 1], f"grow_{gname}")
        nc.sync.dma_start(grow[:], w_in[gname].ap())
        gps = PS([128, 1], small=True)
        nc.tensor.matmul(gps[:], _mm(onesr[:]), _mm(grow[:]),
                         start=True, stop=True)
        gb = T(pvec, [128, 1], f"gbc_{gname}")
        nc.scalar.copy(gb[:], gps[:])
        gbc[gname] = gb

    for wname, gname in zip(COND_WS, GAIN_WS):
        v = T(pvec, [128, NG], f"v_{wname}")
        n2 = T(pPre, [128, NG], "n2", bufs=2)
        std = T(pPre, [128, NG], "nstd", bufs=2)
        inv = T(pvec, [128, NG], f"invn_{wname}")
        for m in range(NG):
            natw = natload(wname, m, 1)
            sq_accum(natw[:, 0, :], n2[:, m:m + 1])
            cscr = T(pPre, [128, C], "cond_scr", bufs=2)
            nc.vector.tensor_mul(cscr[:], natw[:, 0, :], cbc[:])
            nc.vector.tensor_reduce(v[:, m:m + 1], cscr[:],
                                    mybir.AxisListType.X, OP.add)
        nc.scalar.activation(std[:], n2[:], AF.Sqrt)
        nc.vector.reciprocal(inv[:], std[:])
        nc.vector.tensor_mul(v[:], v[:], inv[:])
        nc.vector.tensor_scalar_mul(v[:], v[:], gbc[gname][:])
        invn[wname] = inv
        conds[wname] = v

    one_p_sm = T(pvec, [128, NG], "one_p_sm")
    nc.vector.tensor_scalar_add(one_p_sm[:], conds["sm_scale_W"][:], 1.0)
    one_p_cm = T(pvec, [128, NG], "one_p_cm")
    nc.vector.tensor_scalar_add(one_p_cm[:], conds["cm_scale_W"][:], 1.0)

    # dw taps
    n2dw = T(pPre, [128, 2 * NG], "n2dw", bufs=1)
    for g in range(NG):
        sq_accum(dwh[:, g, :], n2dw[:, g:g + 1])
        sq_accum(dwg[:, g, :], n2dw[:, NG + g:NG + g + 1])
    stddw = T(pPre, [128, 2 * NG], "stddw", bufs=1)
    nc.scalar.activation(stddw[:], n2dw[:], AF.Sqrt)
    invdw = T(pvec, [128, 2 * NG], "invdw")
    nc.vector.reciprocal(invdw[:], stddw[:])
    for g in range(NG):
        nc.vector.tensor_scalar_mul(dwhn[:, g, :], dwh[:, g, :],
                                    invdw[:, g:g + 1])
        nc.vector.tensor_scalar_mul(dwgn[:, g, :], dwg[:, g, :],
                                    invdw[:, NG + g:NG + g + 1])


    af_seq = T(pvec, [128, NG], "af_seq")
    nc.vector.tensor_mul(af_seq[:], conds["sm_alpha_W"][:],
                         invn["seq_out_W"][:])

    def bias_from(lhsT, shift_v, invt, m_tiles, name):
        bias = T(pvec, [128, m_tiles], f"bias_{name}")
        for m in range(m_tiles):
            bps = PS([128, 1], small=True)
            for k in range(len(lhsT)):
                nc.tensor.matmul(bps[:],
                                 lhsT[k][:, m * 128:(m + 1) * 128]
                                 .bitcast(F32),
                                 shift_v[:, k:k + 1],
                                 start=(k == 0), stop=(k == len(lhsT) - 1))
            nc.vector.tensor_scalar_mul(bias[:, m:m + 1], bps[:],
                                        invt[:, m:m + 1])
        return bias

    bias_f = bias_from(lt_fore, conds["sm_shift_W"], invn["fore_W"], 8, "f")
    bias_b = bias_from(lt_back, conds["sm_shift_W"], invn["back_W"], 8, "b")

    def derive(base, invt, name):
        nbi = T(pvec, [128, 8], f"nbias_{name}")
        nc.vector.tensor_scalar_mul(nbi[:], base[:], -1.0)
        b05 = T(pvec, [128, 8], f"b05_{name}")
        nc.vector.tensor_scalar_add(b05[:], base[:], 0.5)
        return nbi, b05

    nbias_f, b05_f = derive(bias_f, invn["fore_W"], "f")
    nbias_b, b05_b = derive(bias_b, invn["back_W"], "b")

    for k in range(NG):
        nc.vector.tensor_scalar_mul(lt_fore[k][:], lt_fore[k][:],
                                    one_p_sm[:, k:k + 1])
        nc.vector.tensor_scalar_mul(lt_back[k][:], lt_back[k][:],
                                    one_p_sm[:, k:k + 1])

    pPre.release()

    # ---------------- pass A1 ----------------
    SfA = T(pA, [128, NG, OV], "SfA")
    HlocC = T(pA, [128, NG, OV], "HlocC")
    cbA = T(pA, [128, NG, 2], "cbA")
    cfC = T(pA, [128, NG, 2], "cfC")
    Pleft = T(pA, [128, NG, 4], "Pleft")
    cy = [T(pA, [128, NG, 1], f"cy{n}") for n in range(5)]

    for ci, (lo, hi) in enumerate(A_CHUNKS if _stageA else []):
        cw = hi - lo
        xt = T(pA, [128, NG, CW], "xA", bufs=1, dt=F32R)
        nc.sync.dma_start(xt[:, :, 0:cw], xdram(lo, hi).bitcast(F32R))
        bps = PS([128, CW])
        nc.tensor.matmul(bps[:, 0:cw], _mm(onesr[:]),
                         _mm(rowAi[:, lo:hi]), start=True, stop=True)
        for g in range(NG):
            nc.vector.tensor_mul(xt[:, g, 0:cw], xt[:, g, 0:cw],
                                 bps[:, 0:cw])
        for dire in ("f", "b"):
            if dire == "f":
                lt, nbi, b05 = lt_fore, nbias_f, b05_f
                invt, bia = invn["fore_W"], bias_f
            else:
                lt, nbi, b05 = lt_back, nbias_b, b05_b
                invt, bia = invn["back_W"], bias_b
            st = T(pA, [128, NG, CW], "sT", bufs=2)
            ct = T(pA, [128, NG, CW], "cT", bufs=2)
            for m in range(8):
                gps = PS([128, CW])
                for k in range(NG):
                    nc.tensor.matmul(
                        gps[:, 0:cw],
                        _mm(lt[k][:, m * 128:(m + 1) * 128]),
                        _mm(xt[:, k, 0:cw]),
                        start=(k == 0), stop=(k == NG - 1))
                if m < 4:
                    nc.scalar.activation(st[:, m, 0:cw], gps[:, 0:cw],
                                         AF.Sigmoid, bias=bia[:, m:m + 1],
                                         scale=invt[:, m:m + 1])
                    nc.vector.tensor_scalar(ct[:, m, 0:cw],
                                            st[:, m, 0:cw],
                                            -1.0, 1.0, OP.mult, OP.add)
                else:
                    mg = m - 4
                    sg = T(pA, [128, CW], "sgA", bufs=1)
                    nc.scalar.activation(sg[:, 0:cw], gps[:, 0:cw],
                                         AF.Sigmoid, bias=bia[:, m:m + 1],
                                         scale=invt[:, m:m + 1])
                    t1 = T(pA, [128, CW], "t1A", bufs=1)
                    nc.vector.tensor_scalar(t1[:, 0:cw], gps[:, 0:cw],
                                            invt[:, m:m + 1],
                                            b05[:, m:m + 1],
                                            OP.mult, OP.add)
                    nc.vector.tensor_max(t1[:, 0:cw], t1[:, 0:cw],
                                         sg[:, 0:cw])
                    nc.vector.tensor_mul(st[:, mg, 0:cw],
                                         st[:, mg, 0:cw], t1[:, 0:cw])
            if dire == "f":
                for g in range(NG):
                    if ci == 0:
                        nc.vector.tensor_tensor_scan(
                            SfA[:, g, :], ct[:, g, 0:cw], st[:, g, 0:cw],
                            0.0, OP.mult, OP.add)
                    elif ci == 1:
                        ini = T(pA, [128, 1], "iniF", bufs=8)
                        nc.vector.tensor_scalar_mul(
                            ini[:], SfA[:, g, OV - 1:OV], selL)
                        nc.vector.tensor_tensor_scan(
                            Hf[:, g, lo - HCOL0:hi - HCOL0],
                            ct[:, g, 0:cw], st[:, g, 0:cw],
                            ini[:], OP.mult, OP.add)
                    elif ci < 5:
                        nc.vector.tensor_tensor_scan(
                            Hf[:, g, lo - HCOL0:hi - HCOL0],
                            ct[:, g, 0:cw], st[:, g, 0:cw],
                            Hf[:, g, lo - HCOL0 - 1:lo - HCOL0],
                            OP.mult, OP.add)
                    else:
                        nc.vector.tensor_copy(cfC[:, g, 0:1],
                                              ct[:, g, 0:1])
                        nc.vector.tensor_copy(cfC[:, g, 1:2],
                                              st[:, g, 0:1])
            else:
                for g in range(NG):
                    if ci == 0:
                        nc.vector.tensor_copy(cbA[:, g, 0:1],
                                              ct[:, g, cw - 1:cw])
                        nc.vector.tensor_copy(cbA[:, g, 1:2],
                                              st[:, g, cw - 1:cw])
                    elif ci < 5:
                        nc.vector.tensor_tensor_scan(
                            Hb[:, g, lo - HCOL0:hi - HCOL0][:, ::-1],
                            ct[:, g, 0:cw][:, ::-1],
                            st[:, g, 0:cw][:, ::-1],
                            0.0, OP.mult, OP.add)
                        pt = T(pA, [128, CW], "Pb", bufs=2)
                        nc.vector.tensor_tensor_scan(
                            pt[:, 0:cw][:, ::-1],
                            ct[:, g, 0:cw][:, ::-1],
                            zeros[:, 0:cw], 1.0, OP.mult, OP.add)
                        nc.vector.tensor_copy(Pleft[:, g, ci - 1:ci],
                                              pt[:, 0:1])
                        nc.sync.dma_start(
                            pspill_t[g * 128:(g + 1) * 128,
                                     lo - OV:hi - OV], pt[:, 0:cw])
                    else:
                        nc.vector.tensor_tensor_scan(
                            HlocC[:, g, :][:, ::-1],
                            ct[:, g, 0:cw][:, ::-1],
                            st[:, g, 0:cw][:, ::-1],
                            0.0, OP.mult, OP.add)

    # boundary columns and fore +1 extension
    for g in (range(NG) if _stageA else []):
        nc.vector.scalar_tensor_tensor(
            Hf[:, g, 2049:2050], cfC[:, g, 0:1], Hf[:, g, 2048:2049],
            cfC[:, g, 1:2], OP.mult, OP.add)
        nc.vector.tensor_copy(Hf[:, g, 0:1], SfA[:, g, OV - 1:OV])
        nc.vector.tensor_copy(Hb[:, g, 2049:2050], HlocC[:, g, 0:1])

    # back carry chain + fixups
    for g in (range(NG) if _stageA else []):
        nc.vector.tensor_scalar_mul(cy[4][:, g, :], HlocC[:, g, 0:1], selR)
        for n in range(4, 0, -1):
            left = (n - 1) * 512 + 1
            nc.vector.scalar_tensor_tensor(
                cy[n - 1][:, g, :], Pleft[:, g, n - 1:n], cy[n][:, g, :],
                Hb[:, g, left:left + 1], OP.mult, OP.add)
    for g in (range(NG) if _stageA else []):
        nc.vector.scalar_tensor_tensor(
            Hb[:, g, 0:1], cbA[:, g, 0:1], cy[0][:, g, :],
            cbA[:, g, 1:2], OP.mult, OP.add)
    pins = {}
    for n in (range(1, 5) if _stageA else []):
        lo, hi = A_CHUNKS[n]
        for g in range(NG):
            pt = T(pA, [128, CW], "PbIn", bufs=2)
            nc.sync.dma_start(pt[:], pspill_t[g * 128:(g + 1) * 128,
                                              lo - OV:hi - OV])
            pins[(n, g)] = pt
    for n in (range(1, 5) if _stageA else []):
        lo, hi = A_CHUNKS[n]
        for g in range(NG):
            nc.vector.scalar_tensor_tensor(
                Hb[:, g, lo - HCOL0:hi - HCOL0], pins[(n, g)][:],
                cy[n][:, g, :],
                Hb[:, g, lo - HCOL0:hi - HCOL0], OP.mult, OP.add)

    if STAGE == "A":
        nc.sync.dma_start(
            out_d.ap().rearrange("(g p) l -> p g l", p=128),
            Hb[:, :, 1:2049])
    pA.release()

    # ---------------- C1: seq_out -> x2 (spilled to DRAM) ----------
    pC1 = tc.alloc_tile_pool(name="pC1p", bufs=1)
    rowB = T(prows, [1, LEXT], "rows", bufs=1)
    rowBi = rowB
    _stageC1 = STAGE in ("C1", "FULL")
    for (lo, hi) in (C1_CHUNKS if _stageC1 else []):
        cw = hi - lo
        co = lo - HCOL0
        xt = T(pC1, [128, NG, CW], "xC", bufs=2)
        nc.sync.dma_start(xt[:, :, 0:cw], xdram(lo, hi))
        x2c = T(pC1, [128, NG, CW], "X2c", bufs=2)
        for m in range(NG):
            sps = PS([128, CW])
            for k in range(8):
                rhs = (Hf[:, k, co:co + cw] if k < 4
                       else Hb[:, k - 4, co:co + cw])
                nc.tensor.matmul(
                    sps[:, 0:cw],
                    _mm(lt_seq[k][:, m * 128:(m + 1) * 128]),
                    _mm(rhs), start=(k == 0), stop=(k == 7))
            nc.vector.scalar_tensor_tensor(
                x2c[:, m, 0:cw], sps[:, 0:cw], af_seq[:, m:m + 1],
                xt[:, m, 0:cw], OP.mult, OP.add)
        nc.sync.dma_start(
            x2spill[:, co:co + cw].rearrange("(g p) l -> p g l", p=128),
            x2c[:, :, 0:cw])
        x2sq = T(pC1, [128, NG, CW], "x2sq", bufs=1)
        nc.scalar.activation(x2sq[:, :, 0:cw], x2c[:, :, 0:cw], AF.Square)
        rps = PS([1, CW], small=True)
        for g in range(NG):
            nc.tensor.matmul(rps[:, 0:cw], _mm(onesc[:]),
                             _mm(x2sq[:, g, 0:cw]),
                             start=(g == 0), stop=(g == NG - 1))
        nc.scalar.copy(rowB[:, co:co + cw], rps[:, 0:cw])
    if _stageC1:
        nc.scalar.activation(rowBi[:, 0:2050], rowB[:, 0:2050], AF.Sqrt,
                             bias=eps[0:1, :], scale=1.0 / D)
        nc.vector.reciprocal(rowBi[:, 0:2050], rowBi[:, 0:2050])
    if STAGE == "C1":
        ocp = T(pC1, [128, NG, CW], "ocp", bufs=2)
        for j in range(4):
            nc.sync.dma_start(
                ocp[:], x2spill[:, 1 + j * 512:1 + (j + 1) * 512].rearrange(
                    "(g p) l -> p g l", p=128))
            nc.sync.dma_start(
                out_d.ap()[:, j * 512:(j + 1) * 512].rearrange(
                    "(g p) l -> p g l", p=128), ocp[:])
    pC1.release()
    # ---------------- late weight prep ----------------
    pWp = tc.alloc_tile_pool(name="pWpp", bufs=1)
    pA2 = pWp  # alias: same transient naming scheme

    def natload2(name, mq, mspan):
        cols = w_in[name].shape[1]
        t = T(pWp, [128, mspan, cols], "nat2", bufs=2)
        nc.sync.dma_start(
            t[:], w_in[name].ap()[mq * 128:(mq + mspan) * 128, :].rearrange(
                "(m p) k -> p m k", p=128))
        return t

    def prep_weight2(name, pool, ltag, m_tiles, k_tiles):
        inv = T(pvec, [128, m_tiles], f"invn_{name}")
        n2 = T(pWp, [128, 8], "n2b", bufs=2)
        std = T(pWp, [128, 8], "nstd2", bufs=2)
        lhsT = [T(pool, [128, m_tiles * 128], ltag, bufs=pool._ltbufs,
                  dt=F32R)
                for _ in range(k_tiles)]
        step = 2 if w_in[name].shape[1] <= 512 else 1
        for mq in range(0, m_tiles, step):
            mspan = min(step, m_tiles - mq)
            natw = natload2(name, mq, mspan)
            for j in range(mspan):
                sq2 = T(pWp, [128, 1024], "sqscr2", bufs=1)
                nc.scalar.activation(sq2[:, 0:natw.shape[-1]],
                                     natw[:, j, :], AF.Square,
                                     accum_out=n2[:, mq + j:mq + j + 1])
                for k in range(k_tiles):
                    tp = PS([128, 128])
                    nc.tensor.transpose(
                        tp[:, 0:128],
                        natw[:, j, k * 128:(k + 1) * 128], ident[:])
                    nc.vector.tensor_copy(
                        lhsT[k][:, (mq + j) * 128:(mq + j + 1) * 128],
                        tp[:, 0:128])
        nc.scalar.activation(std[:, 0:m_tiles], n2[:, 0:m_tiles], AF.Sqrt)
        nc.vector.reciprocal(inv[:], std[:, 0:m_tiles])
        invn[name] = inv
        return lhsT

    _stageC2 = STAGE == "FULL"
    lt_proj = prep_weight2("proj_in_W", pW2k, "lt2k", 4, 4)
    lt_pwh = prep_weight2("pwh_W", pW4k, "lt4k", 8, 4)
    lt_pwg = prep_weight2("pwg_W", pW4k, "lt4k", 8, 4)

    bias_p = T(pvec, [128, NG], "bias_p")
    for m in range(NG):
        bps = PS([128, 1], small=True)
        for k in range(NG):
            nc.tensor.matmul(bps[:],
                             lt_proj[k][:, m * 128:(m + 1) * 128]
                             .bitcast(F32),
                             conds["cm_shift_W"][:, k:k + 1],
                             start=(k == 0), stop=(k == NG - 1))
        nc.vector.tensor_scalar_mul(bias_p[:, m:m + 1], bps[:],
                                    invn["proj_in_W"][:, m:m + 1])
    for k in range(NG):
        nc.vector.tensor_scalar_mul(lt_proj[k][:], lt_proj[k][:],
                                    one_p_cm[:, k:k + 1])

    lt_chn = prep_weight2("chn_out_W", pW2k, "lt2k", 4, 8)

    af_chn = T(pvec, [128, NG], "af_chn")
    nc.vector.tensor_mul(af_chn[:], conds["cm_alpha_W"][:],
                         invn["chn_out_W"][:])
    nc.vector.tensor_scalar_mul(af_chn[:], af_chn[:], 1.0 / 0.596)
    pWp.release()


    # ---------------- C2: fused norm2/proj + dw3/pw/gate/chn/x3 -----
    pC2 = tc.alloc_tile_pool(name="pC2p", bufs=1)
    Rchn = T(pbig, [128, NG, 2050], "big", bufs=2)

    def x2load(co, cols):
        t = T(pC2, [128, NG, 513], "xs", bufs=2, dt=F32R)
        nc.sync.dma_start(
            t[:, :, 0:cols],
            x2spill[:, co:co + cols].rearrange(
                "(g p) l -> p g l", p=128).bitcast(F32R))
        return t

    def front(ci):
        lo, hi = C1_CHUNKS[ci]
        cw = hi - lo
        co = lo - HCOL0
        ld = min(513, 2050 - co)
        x2f = x2load(co, ld)
        bps = PS([128, CW])
        nc.tensor.matmul(bps[:, 0:cw], _mm(onesr[:]),
                         _mm(rowBi[:, co:co + cw]), start=True, stop=True)
        x2h = T(pC2, [128, NG, CW], "x2h", bufs=1, dt=F32R)
        for g in range(NG):
            nc.vector.tensor_mul(x2h[:, g, 0:cw], x2f[:, g, 0:cw],
                                 bps[:, 0:cw])
        for m in range(NG):
            pps = PS([128, CW])
            for k in range(NG):
                nc.tensor.matmul(
                    pps[:, 0:cw],
                    _mm(lt_proj[k][:, m * 128:(m + 1) * 128]),
                    _mm(x2h[:, k, 0:cw]),
                    start=(k == 0), stop=(k == NG - 1))
            nc.scalar.activation(Rchn[:, m, co:co + cw], pps[:, 0:cw],
                                 AF.Identity, bias=bias_p[:, m:m + 1],
                                 scale=invn["proj_in_W"][:, m:m + 1])
        if ci == 0:
            for g in range(NG):
                nc.vector.tensor_scalar_mul(Rchn[:, g, 0:1],
                                            Rchn[:, g, 0:1], selL)
        if ci == len(C1_CHUNKS) - 1:
            for g in range(NG):
                nc.vector.tensor_scalar_mul(Rchn[:, g, 2049:2050],
                                            Rchn[:, g, 2049:2050], selR)
        return x2f

    def backstage(j, x2f):
        lo, hi = C2B_CHUNKS[j]
        cw = hi - lo
        co = lo - HCOL0
        yh = T(pC2, [128, NG, CW], "yh", bufs=1, dt=F32R)
        yg = T(pC2, [128, NG, CW], "yg", bufs=1, dt=F32R)
        for g in range(NG):
            for (yt, wn) in ((yh, dwhn), (yg, dwgn)):
                nc.vector.tensor_scalar_mul(
                    yt[:, g, 0:cw], Rchn[:, g, co - 1:co - 1 + cw],
                    wn[:, g, 0:1])
                nc.vector.scalar_tensor_tensor(
                    yt[:, g, 0:cw], Rchn[:, g, co:co + cw],
                    wn[:, g, 1:2], yt[:, g, 0:cw], OP.mult, OP.add)
                nc.vector.scalar_tensor_tensor(
                    yt[:, g, 0:cw], Rchn[:, g, co + 1:co + 1 + cw],
                    wn[:, g, 2:3], yt[:, g, 0:cw], OP.mult, OP.add)
        hg = T(pC2, [128, 8, CW], "hg", bufs=1, dt=F32R)
        for kk in range(8):
            hps = PS([128, CW])
            gps2 = PS([128, CW])
            for k in range(NG):
                nc.tensor.matmul(
                    hps[:, 0:cw],
                    _mm(lt_pwh[k][:, kk * 128:(kk + 1) * 128]),
                    _mm(yh[:, k, 0:cw]),
                    start=(k == 0), stop=(k == NG - 1))
            for k in range(NG):
                nc.tensor.matmul(
                    gps2[:, 0:cw],
                    _mm(lt_pwg[k][:, kk * 128:(kk + 1) * 128]),
                    _mm(yg[:, k, 0:cw]),
                    start=(k == 0), stop=(k == NG - 1))
            g2 = T(pC2, [128, CW], "g2", bufs=2)
            nc.scalar.activation(g2[:, 0:cw], gps2[:, 0:cw], GATE_FN,
                                 scale=invn["pwg_W"][:, kk:kk + 1])
            nc.vector.scalar_tensor_tensor(
                hg[:, kk, 0:cw], hps[:, 0:cw], invn["pwh_W"][:, kk:kk + 1],
                g2[:, 0:cw], OP.mult, OP.mult)
        ot = T(pC2, [128, NG, CW], "ot", bufs=1)
        for m in range(NG):
            cps = PS([128, CW])
            for kk in range(8):
                nc.tensor.matmul(
                    cps[:, 0:cw],
                    _mm(lt_chn[kk][:, m * 128:(m + 1) * 128]),
                    _mm(hg[:, kk, 0:cw]),
                    start=(kk == 0), stop=(kk == 7))
            nc.vector.scalar_tensor_tensor(
                ot[:, m, 0:cw], cps[:, 0:cw], af_chn[:, m:m + 1],
                x2f[:, m, 1:1 + cw], OP.mult, OP.add)
        nc.sync.dma_start(
            out_d.ap()[:, lo - OV:hi - OV].rearrange(
                "(g p) l -> p g l", p=128), ot[:, :, 0:cw])

    fronts = {}
    for ci in (range(len(C1_CHUNKS)) if _stageC2 else []):
        fronts[ci] = front(ci)
        if ci >= 1:
            backstage(ci - 1, fronts.pop(ci - 1))

    pC2.release()
    prows.release()
    pW2k.release()
    pW4k.release()
    pbig.release()
    pdram.release()
    psum.release()
    pvec.release()
    pconst.release()


@functools.lru_cache(maxsize=1)
def _get_program():
    return build_program()


def make_in_maps(inputs):
    x = np.ascontiguousarray(inputs["x"], dtype=np.float32)
    cfull = np.ascontiguousarray(inputs["c"], dtype=np.float32)
    weights = {}
    for n in MAIN_WS + COND_WS:
        weights[n] = np.ascontiguousarray(inputs[n], dtype=np.float32)
    weights["dwh_W"] = np.ascontiguousarray(
        np.asarray(inputs["dwh_W"]).reshape(D, 3), dtype=np.float32)
    weights["dwg_W"] = np.ascontiguousarray(
        np.asarray(inputs["dwg_W"]).reshape(D, 3), dtype=np.float32)
    for gname in GAIN_WS:
        weights[gname] = np.asarray(inputs[gname],
                                    dtype=np.float32).reshape(1, 1)
    in_maps = []
    for core in range(8):
        b, half = core // 2, core % 2
        start = half * LLOC
        x_ext = np.zeros((D, LEXT), np.float32)
        lo, hi = start - OV, start + LLOC + OV
        slo, shi = max(lo, 0), min(hi, L)
        x_ext[:, slo - lo:shi - lo] = x[b][:, slo:shi]
        selv = np.zeros((128, 2), np.float32)
        selv[:, 0] = 1.0 if half == 1 else 0.0
        selv[:, 1] = 1.0 if half == 0 else 0.0
        m = {"x_ext": x_ext, "c_row": cfull[b:b + 1, :], "sel": selv}
        m.update(weights)
        in_maps.append(m)
    return in_maps


def gather_out(results):
    out = np.zeros((B, D, L), np.float32)
    for core in range(8):
        b, half = core // 2, core % 2
        out[b][:, half * LLOC:(half + 1) * LLOC] = results[core]["out"]
    return out


def kernel(**inputs):
    nc = _get_program()
    in_maps = make_in_maps(inputs)
    res = run_bass_kernel_spmd(nc, in_maps, list(range(8)))
    return gather_out(res.results)



# revision 33
# speedup vs baseline: 1.5269x; 1.5269x over previous
"""Trainium2 Bass kernel for nn_DiTBlock_77979426226864 (v2).

Sharding: 8 cores = (batch b in 0..3) x (sequence half in 0..1); each core
gets a zero-padded extended input x_ext [512, 64+2048+64] and computes its
2048-position output slice. MinGRU halos (64 cols) stand in for cross-core
carries; a 1-col halo feeds the depthwise-3 convs (validated vs reference).

v2 layout/engine plan:
- Weights are staged host-side TRANSPOSED ([K, M]) so lhsT tiles DMA-load
  directly (no on-chip transposes); the gpsimd SWDGE path casts f32->bf16
  during the load. All matmuls run in bf16 (PSUM accumulates f32).
- Per-channel magnitude-preserving norms from the lhsT tiles: square (ACT/
  DVE) + ones-matmul partition reduction into per-m PSUM columns -> one
  Rsqrt ACT. Conditioning scale folds into lhsT columns; shifts fold into
  per-partition sigmoid biases; output norms fold into STT/ACT scales.
- MinGRU is H_t = c_t*H_{t-1} + b_t on DVE tensor_tensor_scan, bf16
  operands. Two chunk sweeps: backward (right->left, reversed APs, chained
  carries) then forward fused with the seq_out matmul (C1). No DRAM
  spills: Hf/Hb/x2/Rchn stay SBUF-resident in bf16.
- Chn-mixer: proj -> dw3 (DVE) -> pwh/pwg matmuls -> silu gate -> chn_out,
  chunked with a 1-chunk lag for the dw3 halo.
"""
import os
import sys
import functools

for _p in ("/opt/trn_rl_repo", "/root/.axon_site"):
    if _p not in sys.path and os.path.isdir(_p):
        sys.path.insert(0, _p)

import numpy as np

import concourse.bass as bass  # noqa: E402
import concourse.bacc as bacc  # noqa: E402
import concourse.tile as tile  # noqa: E402
from concourse import mybir  # noqa: E402
from concourse.bass_utils import run_bass_kernel_spmd  # noqa: E402

F32 = mybir.dt.float32
BF16 = mybir.dt.bfloat16
AF = mybir.ActivationFunctionType
OP = mybir.AluOpType

B, D, L = 4, 512, 4096
C = 256
O = 512
OV = 64
LLOC = L // 2
LEXT = OV + LLOC + OV          # 2176
NG = D // 128                  # 4
CW = 512

# CoreSim lacks Silu; env KERNEL_SIM_SAFE=1 substitutes Sigmoid (debug only)
GATE_FN = (AF.Sigmoid if os.environ.get("KERNEL_SIM_SAFE") else AF.Silu)
# debug bisection: FULL | HF | HB | X2 | RCHN (dump intermediate to out)
STAGE = os.environ.get("KERNEL_STAGE", "FULL")

# main weights: name -> (transposed dram shape [K, M])
MAIN_WS = {"fore_W": (512, 1024), "back_W": (512, 1024),
           "seq_out_W": (1024, 512), "proj_in_W": (512, 512),
           "pwh_W": (512, 1024), "pwg_W": (512, 1024),
           "chn_out_W": (1024, 512)}
COND_WS = ["sm_scale_W", "sm_shift_W", "sm_alpha_W",
           "cm_scale_W", "cm_shift_W", "cm_alpha_W"]
GAIN_WS = ["sm_scale_g", "sm_shift_g", "sm_alpha_g",
           "cm_scale_g", "cm_shift_g", "cm_alpha_g"]


def build_program():
    nc = bacc.Bacc("TRN2", target_bir_lowering=False, debug=False,
                   num_devices=8)

    x_in = nc.dram_tensor("x_ext", [D, LEXT], F32, kind="ExternalInput")
    c_in = nc.dram_tensor("c_col", [C, 1], F32, kind="ExternalInput")
    sel_in = nc.dram_tensor("sel", [128, 2], F32, kind="ExternalInput")
    w_in = {}
    for n, (k, m) in MAIN_WS.items():
        w_in[n] = nc.dram_tensor(n + "t", [k, m], F32, kind="ExternalInput")
    for n in COND_WS:
        w_in[n] = nc.dram_tensor(n + "t", [C, D], F32, kind="ExternalInput")
    for n in GAIN_WS:
        w_in[n] = nc.dram_tensor(n, [1, 1], F32, kind="ExternalInput")
    w_in["dwh_W"] = nc.dram_tensor("dwh_W", [D, 3], F32,
                                   kind="ExternalInput")
    w_in["dwg_W"] = nc.dram_tensor("dwg_W", [D, 3], F32,
                                   kind="ExternalInput")
    out_d = nc.dram_tensor("out", [D, LLOC], F32, kind="ExternalOutput")

    onesc_d = nc.inline_tensor(np.ones((128, 1), np.float32), name="onescol")
    onesr_d = nc.inline_tensor(np.ones((1, 128), np.float32), name="onesrow")

    with tile.TileContext(nc) as tc, nc.allow_low_precision(
            reason="bf16 datapath validated against reference (2e-2 budget)"):
        _emit(nc, tc, x_in, c_in, sel_in, w_in, out_d, onesc_d, onesr_d)
    nc.compile()
    return nc


def _emit(nc, tc, x_in, c_in, sel_in, w_in, out_d, onesc_d, onesr_d):

    def xdram(lo, hi):
        return x_in.ap()[:, lo:hi].rearrange("(g p) l -> p g l", p=128)

    # ---------------- pools (strict LIFO release order) ----------------
    pconst = tc.alloc_tile_pool(name="constp", bufs=1)
    pW = tc.alloc_tile_pool(name="wp", bufs=1)
    pbig = tc.alloc_tile_pool(name="bigp", bufs=1)
    prow = tc.alloc_tile_pool(name="rowp", bufs=1)
    psum = tc.alloc_tile_pool(name="psump", bufs=1, space="PSUM")

    def T(pool, shape, tag, bufs=1, dt=BF16):
        return pool.tile(shape, dt, tag=tag, bufs=bufs, name=tag)

    def PS(shape, small=False):
        return psum.tile(shape, F32, tag="psS" if small else "psB",
                         bufs=2 if small else 6,
                         name="psS" if small else "psB")

    # ---------------- constants ----------------
    onescf = T(pconst, [128, 1], "onescf", dt=F32)
    nc.scalar.dma_start(onescf[:], onesc_d.ap())
    onesrf = T(pconst, [1, 128], "onesrf", dt=F32)
    nc.scalar.dma_start(onesrf[:], onesr_d.ap())
    onescb = T(pconst, [128, 1], "onescb")
    nc.vector.tensor_copy(onescb[:], onescf[:])
    eps = T(pconst, [128, 1], "eps", dt=F32)
    nc.gpsimd.memset(eps[:], 1e-4)
    sel = T(pconst, [128, 2], "sel", dt=F32)
    nc.scalar.dma_start(sel[:], sel_in.ap())
    selL, selR = sel[:, 0:1], sel[:, 1:2]
    dwh = T(pconst, [128, NG, 3], "dwh", dt=F32)
    nc.scalar.dma_start(dwh[:], w_in["dwh_W"].ap().rearrange(
        "(g p) k -> p g k", p=128))
    dwg = T(pconst, [128, NG, 3], "dwg", dt=F32)
    nc.scalar.dma_start(dwg[:], w_in["dwg_W"].ap().rearrange(
        "(g p) k -> p g k", p=128))
    dwhn = T(pconst, [128, NG, 3], "dwhn", dt=F32)
    dwgn = T(pconst, [128, NG, 3], "dwgn", dt=F32)
    cbf = T(pconst, [128, 2, 1], "cbf")
    nc.gpsimd.dma_start(cbf[:], c_in.ap().rearrange("(q p) o -> p q o",
                                                    p=128))

    # ---------------- weight loads (cast f32->bf16 in SWDGE DMA) -------
    # queue order matters on the SWDGE engine: cond weights (small, gate
    # the bias/cond chain) first, then gate weights, then the rest.
    lt = {}
    for n in MAIN_WS:
        k, m = MAIN_WS[n]
        lt[n] = T(pW, [128, k // 128, m], f"lt_{n}")
    for n in ("fore_W", "back_W"):
        nc.gpsimd.dma_start(
            lt[n][:], w_in[n].ap().rearrange("(q p) m -> p q m", p=128))
    ltc = {}
    for n in COND_WS:
        t = T(pW, [128, 2, D], f"ltc_{n}")
        nc.gpsimd.dma_start(
            t[:], w_in[n].ap().rearrange("(q p) m -> p q m", p=128))
        ltc[n] = t

    # ---------------- persistent big tiles ----------------
    Hf = T(pbig, [128, NG, 2052], "Hf")
    Hb = T(pbig, [128, NG, 2052], "Hb")
    x2r = T(pbig, [128, NG, 2050], "x2r")
    Rchn = T(pbig, [128, NG, 2050], "Rchn")
    xnr = T(pbig, [128, NG, LEXT], "xnr")
    rowA = T(prow, [1, LEXT], "rowA", dt=F32)
    rowB = T(prow, [1, LEXT], "rowB", dt=F32)
    pdram = tc.alloc_tile_pool(name="dramp", bufs=1, space="DRAM")
    drow = pdram.tile([1, LEXT], F32, tag="drow", bufs=2, name="drow")

    def row_invsqrt(row, pool, lo, klen):
        """row[lo:lo+128*klen] := 1/sqrt(row/D + eps), computed in a
        [128,klen] layout via a DRAM roundtrip (parallel across
        partitions; off the 1-partition serial path)."""
        n = 128 * klen
        dslice = drow[:, lo:lo + n]
        nc.scalar.dma_start(dslice, row[:, lo:lo + n])
        rseg = T(pool, [128, 17], "rseg", bufs=2, dt=F32)
        nc.scalar.dma_start(rseg[:, 0:klen],
                            dslice.rearrange("o (p k) -> (o p) k", p=128))
        nc.scalar.activation(rseg[:, 0:klen], rseg[:, 0:klen], AF.Sqrt,
                             bias=eps[:, 0:1], scale=1.0 / D)
        nc.vector.reciprocal(rseg[:, 0:klen], rseg[:, 0:klen])
        nc.scalar.dma_start(
            dslice.rearrange("o (p k) -> (o p) k", p=128), rseg[:, 0:klen])
        nc.scalar.dma_start(row[:, lo:lo + n], dslice)

    # ------------- stats pre-pass (needs only x; runs during prep) ------
    pSt = tc.alloc_tile_pool(name="statp", bufs=1)
    for (slo, shi) in ((2048, 2176), (1536, 2048), (1024, 1536),
                       (512, 1024), (0, 512)):
        scw = shi - slo
        xts = T(pSt, [128, NG, CW], "xts", bufs=2, dt=F32)
        nc.sync.dma_start(xts[:, :, 0:scw], xdram(slo, shi))
        sqs = T(pSt, [128, NG, CW], "sqs", bufs=2)
        nc.vector.tensor_mul(sqs[:, :, 0:scw], xts[:, :, 0:scw],
                             xts[:, :, 0:scw])
        rps = PS([1, CW])
        for g in range(NG):
            nc.tensor.matmul(rps[:, 0:scw], onescb[:], sqs[:, g, 0:scw],
                             start=(g == 0), stop=(g == NG - 1))
        nc.vector.tensor_copy(rowA[:, slo:shi], rps[:, 0:scw])
        # finalize inverse-std pieces as soon as their raw sums land,
        # right-to-left to match sweep B's consumption order
        if slo == 2048:
            row_invsqrt(rowA, pSt, 2048, 1)
        elif slo == 1024:
            row_invsqrt(rowA, pSt, 1024, 8)
        elif slo == 0:
            row_invsqrt(rowA, pSt, 0, 8)

    # gains -> [128, 1] broadcasts
    gb = {}
    for gname in GAIN_WS:
        grow = T(pSt, [1, 1], f"grow_{gname}", dt=F32)
        nc.scalar.dma_start(grow[:], w_in[gname].ap())
        gps = PS([128, 1], small=True)
        nc.tensor.matmul(gps[:], onesrf[:], grow[:], start=True, stop=True)
        g = T(pconst, [128, 1], f"gb_{gname}", dt=F32)
        nc.scalar.copy(g[:], gps[:])
        gb[gname] = g

    # conditioning vectors: v = gain * invnorm * (W.T-tiles @ c)
    conds = {}
    for wname, gname in zip(COND_WS, GAIN_WS):
        ltcw = ltc[wname]
        sqc = T(pSt, [128, 2, D], "sqc", bufs=2)
        for q in range(2):
            nc.scalar.activation(sqc[:, q, :], ltcw[:, q, :], AF.Square)
        n2ps = PS([128, NG], small=True)
        vps = PS([128, NG], small=True)
        for mb in range(NG):
            for q in range(2):
                nc.tensor.matmul(n2ps[:, mb:mb + 1],
                                 sqc[:, q, mb * 128:(mb + 1) * 128],
                                 onescb[:], start=(q == 0), stop=(q == 1))
        for mb in range(NG):
            for q in range(2):
                nc.tensor.matmul(vps[:, mb:mb + 1],
                                 ltcw[:, q, mb * 128:(mb + 1) * 128],
                                 cbf[:, q, :], start=(q == 0), stop=(q == 1))
        invc = T(pSt, [128, NG], "invc", bufs=2, dt=F32)
        nc.scalar.activation(invc[:], n2ps[:], AF.Sqrt)
        nc.vector.reciprocal(invc[:], invc[:])
        v = T(pconst, [128, NG], f"v_{wname}", dt=F32)
        nc.vector.tensor_mul(v[:], vps[:], invc[:])
        nc.vector.tensor_scalar_mul(v[:], v[:], gb[gname][:])
        conds[wname] = v
    pSt.release()

    # ---------------- prep pool ----------------
    pPre = tc.alloc_tile_pool(name="prep", bufs=1)

    # norms: square + ones-matmul partition reduce (gate weights now;
    # the rest after their DMAs are emitted, post sweep-B)
    invn = {}

    def weight_norm(n, pool):
        k, m = MAIN_WS[n]
        kt, mt = k // 128, m // 128
        n2ps = PS([128, 32], small=True)
        for q in range(kt):
            sq = T(pool, [128, m], "sqw", bufs=1)
            if n in ("fore_W", "back_W", "pwh_W", "pwg_W"):
                nc.scalar.activation(sq[:], lt[n][:, q, :], AF.Square)
            else:
                nc.vector.tensor_mul(sq[:], lt[n][:, q, :], lt[n][:, q, :])
            for mb in range(mt):
                nc.tensor.matmul(n2ps[:, mb * kt + q:mb * kt + q + 1],
                                 sq[:, mb * 128:(mb + 1) * 128],
                                 onescb[:], start=True, stop=True)
        inv = T(pconst, [128, mt], f"invn_{n}", dt=F32)
        for mb in range(mt):
            nc.vector.tensor_reduce(inv[:, mb:mb + 1],
                                    n2ps[:, mb * kt:(mb + 1) * kt],
                                    mybir.AxisListType.X, OP.add)
        invn[n] = inv
        return inv

    def finish_norms(names):
        for n in names:
            inv = invn[n]
            nc.scalar.activation(inv[:], inv[:], AF.Sqrt)
            nc.vector.reciprocal(inv[:], inv[:])

    weight_norm("fore_W", pPre)
    weight_norm("back_W", pPre)
    finish_norms(("fore_W", "back_W"))

    one_p_sm = T(pconst, [128, NG], "one_p_sm", dt=F32)
    nc.vector.tensor_scalar_add(one_p_sm[:], conds["sm_scale_W"][:], 1.0)
    one_p_cm = T(pconst, [128, NG], "one_p_cm", dt=F32)
    nc.vector.tensor_scalar_add(one_p_cm[:], conds["cm_scale_W"][:], 1.0)

    # dw tap norms
    n2dw = T(pPre, [128, 2 * NG], "n2dw", dt=F32)
    sqd = T(pPre, [128, 3], "sqd", dt=F32)
    for g in range(NG):
        nc.scalar.activation(sqd[:], dwh[:, g, :], AF.Square,
                             accum_out=n2dw[:, g:g + 1])
        nc.scalar.activation(sqd[:], dwg[:, g, :], AF.Square,
                             accum_out=n2dw[:, NG + g:NG + g + 1])
    invdw = T(pPre, [128, 2 * NG], "invdw", dt=F32)
    nc.scalar.activation(invdw[:], n2dw[:], AF.Sqrt)
    nc.vector.reciprocal(invdw[:], invdw[:])
    for g in range(NG):
        nc.vector.tensor_scalar_mul(dwhn[:, g, :], dwh[:, g, :],
                                    invdw[:, g:g + 1])
        nc.vector.tensor_scalar_mul(dwgn[:, g, :], dwg[:, g, :],
                                    invdw[:, NG + g:NG + g + 1])

    # shift vectors in bf16 (rhs for bias matmuls)
    shsm = T(pconst, [128, NG], "shsm")
    nc.vector.tensor_copy(shsm[:], conds["sm_shift_W"][:])
    shcm = T(pconst, [128, NG], "shcm")
    nc.vector.tensor_copy(shcm[:], conds["cm_shift_W"][:])

    def bias_from(ltw, shift_bf, invt, m_tiles, name):
        bias = T(pconst, [128, m_tiles], f"bias_{name}", dt=F32)
        bps = PS([128, m_tiles], small=True)
        for mb in range(m_tiles):
            for q in range(NG):
                nc.tensor.matmul(bps[:, mb:mb + 1],
                                 ltw[:, q, mb * 128:(mb + 1) * 128],
                                 shift_bf[:, q:q + 1],
                                 start=(q == 0), stop=(q == NG - 1))
        nc.vector.tensor_mul(bias[:], bps[:], invt[:])
        return bias

    bias_f = bias_from(lt["fore_W"], shsm, invn["fore_W"], 8, "f")
    bias_b = bias_from(lt["back_W"], shsm, invn["back_W"], 8, "b")

    b05_f = T(pconst, [128, 8], "b05_f", dt=F32)
    nc.vector.tensor_scalar_add(b05_f[:], bias_f[:], 0.5)
    b05_b = T(pconst, [128, 8], "b05_b", dt=F32)
    nc.vector.tensor_scalar_add(b05_b[:], bias_b[:], 0.5)

    # fold (1 + scale) into gate lhsT columns (per input channel)
    for q in range(NG):
        nc.vector.tensor_scalar_mul(lt["fore_W"][:, q, :],
                                    lt["fore_W"][:, q, :],
                                    one_p_sm[:, q:q + 1])
        nc.vector.tensor_scalar_mul(lt["back_W"][:, q, :],
                                    lt["back_W"][:, q, :],
                                    one_p_sm[:, q:q + 1])
    pPre.release()

    # ---------------- sweep pool + helpers ----------------
    pSw = tc.alloc_tile_pool(name="swp", bufs=1)

    def loadB(lo, hi):
        """DMA x chunk (bf16 cast) + write normalized xn into xnr."""
        cw = hi - lo
        xt = T(pSw, [128, NG, 514], "xt", bufs=2)
        nc.gpsimd.dma_start(xt[:, :, 0:cw], xdram(lo, hi))
        bps = PS([128, CW])
        nc.tensor.matmul(bps[:, 0:cw], onesrf[:], rowA[:, lo:hi],
                         start=True, stop=True)
        bpsb = T(pSw, [128, CW], "bpsb", bufs=1)
        nc.scalar.copy(bpsb[:, 0:cw], bps[:, 0:cw])
        for g in range(NG):
            nc.vector.tensor_mul(xnr[:, g, lo:hi], xt[:, g, 0:cw],
                                 bpsb[:, 0:cw])
        return xt

    def loadF(lo, cw):
        xt = T(pSw, [128, NG, 514], "xtF", bufs=2)
        nc.gpsimd.dma_start(xt[:, :, 0:cw], xdram(lo, lo + cw))
        return xt

    def gates(xlo, cw, dire):
        """ct, st tiles [128, NG, cw] bf16 for direction dire, reading
        resident xnr[:, :, xlo:xlo+cw]."""
        if dire == "f":
            ltw, bia, inv, b05 = lt["fore_W"], bias_f, invn["fore_W"], b05_f
        else:
            ltw, bia, inv, b05 = lt["back_W"], bias_b, invn["back_W"], b05_b
        st = T(pSw, [128, NG, CW], "stg", bufs=2)
        ct = T(pSw, [128, NG, CW], "ctg", bufs=2)
        for m in range(8):
            gps = PS([128, CW])
            for q in range(NG):
                nc.tensor.matmul(gps[:, 0:cw],
                                 ltw[:, q, m * 128:(m + 1) * 128],
                                 xnr[:, q, xlo:xlo + cw],
                                 start=(q == 0), stop=(q == NG - 1))
            if m < 4:
                nc.scalar.activation(st[:, m, 0:cw], gps[:, 0:cw],
                                     AF.Sigmoid, bias=bia[:, m:m + 1],
                                     scale=inv[:, m:m + 1])
                nc.vector.tensor_scalar(ct[:, m, 0:cw], st[:, m, 0:cw],
                                        -1.0, 1.0, OP.mult, OP.add)
            else:
                mg = m - 4
                sg = T(pSw, [128, CW], "sg", bufs=1)
                nc.scalar.activation(sg[:, 0:cw], gps[:, 0:cw], AF.Sigmoid,
                                     bias=bia[:, m:m + 1],
                                     scale=inv[:, m:m + 1])
                t1 = T(pSw, [128, CW], "t1", bufs=1)
                nc.scalar.activation(t1[:, 0:cw], gps[:, 0:cw],
                                     AF.Identity,
                                     bias=b05[:, m:m + 1],
                                     scale=inv[:, m:m + 1])
                nc.vector.tensor_max(t1[:, 0:cw], t1[:, 0:cw], sg[:, 0:cw])
                nc.vector.tensor_mul(st[:, mg, 0:cw], st[:, mg, 0:cw],
                                     t1[:, 0:cw])
        return ct, st

    # ======== sweep B: right halo, owned right->left, left tail ========
    loadB(2112, 2176)
    ct, st = gates(2112, 64, "b")
    HloC = T(pSw, [128, NG, 64], "HloC")
    for g in range(NG):
        nc.vector.tensor_tensor_scan(
            HloC[:, g, :][:, ::-1], ct[:, g, 0:64][:, ::-1],
            st[:, g, 0:64][:, ::-1], 0.0, OP.mult, OP.add)
    iniB = T(pSw, [128, NG, 1], "iniB", dt=F32)
    for g in range(NG):
        nc.vector.tensor_copy(Hb[:, g, 2050:2051], HloC[:, g, 0:1])
        nc.vector.tensor_scalar_mul(iniB[:, g, :], HloC[:, g, 0:1], selR)

    carB = iniB
    for ci, lo in enumerate((1600, 1088, 576, 64)):
        loadB(lo, lo + 512)
        ct, st = gates(lo, 512, "b")
        a = lo - 62
        nxt = T(pSw, [128, NG, 1], "carB", bufs=2, dt=F32)
        for g in range(NG):
            nc.vector.tensor_tensor_scan(
                Hb[:, g, a:a + 512][:, ::-1], ct[:, g, 0:512][:, ::-1],
                st[:, g, 0:512][:, ::-1], carB[:, g, :], OP.mult, OP.add)
            nc.vector.tensor_copy(nxt[:, g, :], Hb[:, g, a:a + 1])
        carB = nxt

    # left tail [0, 64): back 1-col extension + fore halo warmup
    loadB(0, 64)
    ct, st = gates(0, 64, "b")
    for g in range(NG):
        nc.vector.scalar_tensor_tensor(
            Hb[:, g, 1:2], ct[:, g, 63:64], Hb[:, g, 2:3],
            st[:, g, 63:64], OP.mult, OP.add)
    ctf, stf = gates(0, 64, "f")
    Hsf = T(pSw, [128, NG, 64], "Hsf")
    iniF = T(pSw, [128, NG, 1], "iniF", dt=F32)
    for g in range(NG):
        nc.vector.tensor_tensor_scan(
            Hsf[:, g, :], ctf[:, g, 0:64], stf[:, g, 0:64],
            0.0, OP.mult, OP.add)
        nc.vector.tensor_copy(Hf[:, g, 1:2], Hsf[:, g, 63:64])
        nc.vector.tensor_scalar_mul(iniF[:, g, :], Hsf[:, g, 63:64], selL)

    # ---- remaining weights: DMA (behind sweep-B x loads) + prep ----
    for n in ("seq_out_W", "proj_in_W", "pwh_W", "pwg_W", "chn_out_W"):
        nc.gpsimd.dma_start(
            lt[n][:], w_in[n].ap().rearrange("(q p) m -> p q m", p=128))
    pPre2 = tc.alloc_tile_pool(name="prep2", bufs=1)
    for n in ("seq_out_W", "proj_in_W", "pwh_W", "pwg_W", "chn_out_W"):
        weight_norm(n, pPre2)
    finish_norms(("seq_out_W", "proj_in_W", "pwh_W", "pwg_W", "chn_out_W"))
    bias_p = bias_from(lt["proj_in_W"], shcm, invn["proj_in_W"], 4, "p")
    for q in range(NG):
        nc.vector.tensor_scalar_mul(lt["proj_in_W"][:, q, :],
                                    lt["proj_in_W"][:, q, :],
                                    one_p_cm[:, q:q + 1])
    af_seq = T(pconst, [128, NG], "af_seq", dt=F32)
    nc.vector.tensor_mul(af_seq[:], conds["sm_alpha_W"][:],
                         invn["seq_out_W"][:])
    af_chn = T(pconst, [128, NG], "af_chn", dt=F32)
    nc.vector.tensor_mul(af_chn[:], conds["cm_alpha_W"][:],
                         invn["chn_out_W"][:])
    nc.vector.tensor_scalar_mul(af_chn[:], af_chn[:], 1.0 / 0.596)
    pPre2.release()

    # ======== sweep F: forward + fused C1 ========
    def c1_chunk(j0, cw, xt, xoff):
        """x2 cols [j0, j0+cw) from Hf/Hb + residual from xt."""
        for m in range(NG):
            sps = PS([128, CW]) if cw > 2 else PS([128, 2], small=True)
            for k in range(8):
                rhs = (Hf[:, k, j0 + 1:j0 + 1 + cw] if k < 4
                       else Hb[:, k - 4, j0 + 1:j0 + 1 + cw])
                nc.tensor.matmul(
                    sps[:, 0:cw],
                    lt["seq_out_W"][:, k, m * 128:(m + 1) * 128],
                    rhs, start=(k == 0), stop=(k == 7))
            nc.vector.scalar_tensor_tensor(
                x2r[:, m, j0:j0 + cw], sps[:, 0:cw], af_seq[:, m:m + 1],
                xt[:, m, xoff:xoff + cw], OP.mult, OP.add)
        sq = T(pSw, [128, NG, CW], "sqx", bufs=1)
        nc.vector.tensor_mul(sq[:, :, 0:cw], x2r[:, :, j0:j0 + cw],
                             x2r[:, :, j0:j0 + cw])
        rps = PS([1, CW])
        for g in range(NG):
            nc.tensor.matmul(rps[:, 0:cw], onescb[:], sq[:, g, 0:cw],
                             start=(g == 0), stop=(g == NG - 1))
        nc.scalar.copy(rowB[:, j0:j0 + cw], rps[:, 0:cw])

    def front(j0, cw, pool):
        bps = PS([128, CW])
        nc.tensor.matmul(bps[:, 0:cw], onesrf[:], rowB[:, j0:j0 + cw],
                         start=True, stop=True)
        bpsb = T(pool, [128, CW],
                 "bpsb" if pool is pSw else "bpsbF", bufs=1)
        nc.scalar.copy(bpsb[:, 0:cw], bps[:, 0:cw])
        x2n = T(pool, [128, NG, CW],
                "ctg" if pool is pSw else "x2n", bufs=2 if pool is pSw
                else 1)
        for g in range(NG):
            nc.vector.tensor_mul(x2n[:, g, 0:cw], x2r[:, g, j0:j0 + cw],
                                 bpsb[:, 0:cw])
        for m in range(NG):
            pps = PS([128, CW]) if cw > 2 else PS([128, 2], small=True)
            for q in range(NG):
                nc.tensor.matmul(
                    pps[:, 0:cw],
                    lt["proj_in_W"][:, q, m * 128:(m + 1) * 128],
                    x2n[:, q, 0:cw], start=(q == 0), stop=(q == NG - 1))
            nc.scalar.activation(Rchn[:, m, j0:j0 + cw], pps[:, 0:cw],
                                 AF.Identity, bias=bias_p[:, m:m + 1],
                                 scale=invn["proj_in_W"][:, m:m + 1])


    xt3 = None
    carF = iniF
    for i in range(4):
        lo = 64 + 512 * i
        cw = 514 if i == 3 else 513
        xt = loadF(lo - 1, cw)
        ct, st = gates(lo, 512, "f")
        a = 512 * i + 2
        nxt = T(pSw, [128, NG, 1], "carF", bufs=2, dt=F32)
        for g in range(NG):
            nc.vector.tensor_tensor_scan(
                Hf[:, g, a:a + 512], ct[:, g, 0:512], st[:, g, 0:512],
                carF[:, g, :], OP.mult, OP.add)
            nc.vector.tensor_copy(nxt[:, g, :], Hf[:, g, a + 511:a + 512])
        carF = nxt
        if i == 3:
            xt3 = xt
            # 1-col fore extension at ext col 2112 (xnr resident there)
            ctf1, stf1 = gates(2112, 1, "f")
            for g in range(NG):
                nc.vector.scalar_tensor_tensor(
                    Hf[:, g, 2050:2051], ctf1[:, g, 0:1],
                    Hf[:, g, 2049:2050], stf1[:, g, 0:1],
                    OP.mult, OP.add)
        c1_chunk(512 * i, 512, xt, 0)
        if i == 1:
            row_invsqrt(rowB, pSw, 0, 8)
        elif i == 2:
            front(0, 512, pSw)
            for g in range(NG):
                nc.vector.tensor_scalar_mul(Rchn[:, g, 0:1],
                                            Rchn[:, g, 0:1], selL)
        elif i == 3:
            row_invsqrt(rowB, pSw, 1024, 8)
            front(512, 512, pSw)
    # C1 edge: x2 cols [2048, 2050) (uses xt3 cols 512..514)
    c1_chunk(2048, 2, xt3, 512)
    row_invsqrt(rowB, pSw, 2048, 1)
    pSw.release()

    # ======== C2 ========
    pC2 = tc.alloc_tile_pool(name="c2p", bufs=1)

    def backstage(out_lo, cols):
        c0 = out_lo + 1
        yh = T(pC2, [128, NG, CW], "yh")
        yg = T(pC2, [128, NG, CW], "yg")
        for g in range(NG):
            for (yt, wn) in ((yh, dwhn), (yg, dwgn)):
                nc.vector.tensor_scalar_mul(
                    yt[:, g, 0:cols], Rchn[:, g, c0 - 1:c0 - 1 + cols],
                    wn[:, g, 0:1])
                nc.vector.scalar_tensor_tensor(
                    yt[:, g, 0:cols], Rchn[:, g, c0:c0 + cols],
                    wn[:, g, 1:2], yt[:, g, 0:cols], OP.mult, OP.add)
                nc.vector.scalar_tensor_tensor(
                    yt[:, g, 0:cols], Rchn[:, g, c0 + 1:c0 + 1 + cols],
                    wn[:, g, 2:3], yt[:, g, 0:cols], OP.mult, OP.add)
        hg = T(pC2, [128, 8, CW], "hg")
        for kk in range(8):
            hps = PS([128, CW])
            gps2 = PS([128, CW])
            for q in range(NG):
                nc.tensor.matmul(hps[:, 0:cols],
                                 lt["pwh_W"][:, q, kk * 128:(kk + 1) * 128],
                                 yh[:, q, 0:cols], start=(q == 0),
                                 stop=(q == NG - 1))
            for q in range(NG):
                nc.tensor.matmul(gps2[:, 0:cols],
                                 lt["pwg_W"][:, q, kk * 128:(kk + 1) * 128],
                                 yg[:, q, 0:cols], start=(q == 0),
                                 stop=(q == NG - 1))
            g2 = T(pC2, [128, CW], "g2", bufs=2)
            nc.scalar.activation(g2[:, 0:cols], gps2[:, 0:cols], GATE_FN,
                                 scale=invn["pwg_W"][:, kk:kk + 1])
            nc.vector.scalar_tensor_tensor(
                hg[:, kk, 0:cols], hps[:, 0:cols],
                invn["pwh_W"][:, kk:kk + 1], g2[:, 0:cols],
                OP.mult, OP.mult)
        ot = T(pC2, [128, NG, CW], "ot", bufs=1, dt=F32)
        for m in range(NG):
            cps = PS([128, CW])
            for kk in range(8):
                nc.tensor.matmul(
                    cps[:, 0:cols],
                    lt["chn_out_W"][:, kk, m * 128:(m + 1) * 128],
                    hg[:, kk, 0:cols], start=(kk == 0), stop=(kk == 7))
            nc.vector.scalar_tensor_tensor(
                ot[:, m, 0:cols], cps[:, 0:cols], af_chn[:, m:m + 1],
                x2r[:, m, c0:c0 + cols], OP.mult, OP.add)
        nc.sync.dma_start(
            out_d.ap()[:, out_lo:out_lo + cols].rearrange(
                "(g p) l -> p g l", p=128), ot[:, :, 0:cols])

    if STAGE in ("HF", "HB", "X2"):
        dbg = {"HF": Hf, "HB": Hb, "X2": x2r}[STAGE]
        ofs = 1 if STAGE == "X2" else 2
        dbt = T(pC2, [128, NG, CW], "dbt", bufs=2, dt=F32)
        for j in range(4):
            for g in range(NG):
                nc.vector.tensor_copy(
                    dbt[:, g, :],
                    dbg[:, g, ofs + 512 * j:ofs + 512 + 512 * j])
            nc.sync.dma_start(
                out_d.ap()[:, 512 * j:512 * (j + 1)].rearrange(
                    "(g p) l -> p g l", p=128), dbt[:])
    front(0, 512)
    for g in range(NG):
        nc.vector.tensor_scalar_mul(Rchn[:, g, 0:1], Rchn[:, g, 0:1], selL)
    front(512, 512)
    backstage(0)
    front(1024, 512)
    backstage(1)
    front(1536, 512)
    backstage(2)
    front(2048, 2)
    for g in range(NG):
        nc.vector.tensor_scalar_mul(Rchn[:, g, 2049:2050],
                                    Rchn[:, g, 2049:2050], selR)
    backstage(3)

    pC2.release()
    pdram.release()
    psum.release()
    prow.release()
    pbig.release()
    pW.release()
    pconst.release()


@functools.lru_cache(maxsize=1)
def _get_program():
    return build_program()


def make_in_maps(inputs):
    x = np.ascontiguousarray(inputs["x"], dtype=np.float32)
    cfull = np.ascontiguousarray(inputs["c"], dtype=np.float32)
    weights = {}
    for n in MAIN_WS:
        weights[n + "t"] = np.ascontiguousarray(
            np.asarray(inputs[n], dtype=np.float32).T)
    for n in COND_WS:
        weights[n + "t"] = np.ascontiguousarray(
            np.asarray(inputs[n], dtype=np.float32).T)
    weights["dwh_W"] = np.ascontiguousarray(
        np.asarray(inputs["dwh_W"]).reshape(D, 3), dtype=np.float32)
    weights["dwg_W"] = np.ascontiguousarray(
        np.asarray(inputs["dwg_W"]).reshape(D, 3), dtype=np.float32)
    for gname in GAIN_WS:
        weights[gname] = np.asarray(inputs[gname],
                                    dtype=np.float32).reshape(1, 1)
    in_maps = []
    for core in range(8):
        b, half = core // 2, core % 2
        start = half * LLOC
        x_ext = np.zeros((D, LEXT), np.float32)
        lo, hi = start - OV, start + LLOC + OV
        slo, shi = max(lo, 0), min(hi, L)
        x_ext[:, slo - lo:shi - lo] = x[b][:, slo:shi]
        selv = np.zeros((128, 2), np.float32)
        selv[:, 0] = 1.0 if half == 1 else 0.0
        selv[:, 1] = 1.0 if half == 0 else 0.0
        m = {"x_ext": x_ext, "c_col": cfull[b].reshape(C, 1), "sel": selv}
        m.update(weights)
        in_maps.append(m)
    return in_maps


def gather_out(results):
    out = np.zeros((B, D, L), np.float32)
    for core in range(8):
        b, half = core // 2, core % 2
        out[b][:, half * LLOC:(half + 1) * LLOC] = results[core]["out"]
    return out


def kernel(**inputs):
    nc = _get_program()
    in_maps = make_in_maps(inputs)
    res = run_bass_kernel_spmd(nc, in_maps, list(range(8)))
    return gather_out(res.results)
